# revision 44
# baseline (speedup 1.0000x reference)
"""Trainium2 Bass kernel for nn_Memory_45964740002665 (scatter_memory).

Distribution (8 NeuronCores):
  * Update phase sharded by memory-slot block: core c owns slots
    [576c, 576c+576). It computes score[8192 q, 576 slots] (f32r matmuls),
    per-query local row-max, local one-hot m0 = (score >= rowmax_local)
    stored bf16, and the per-slot column-max (on GPSIMD). A 32KB
    AllReduce(max) yields the global row-max; queries whose max lives on
    another core are zeroed via the per-query scale
    qscale = rnorm * exp(rowmax) * (rowmax_local >= rowmax_global),
    so qu^T = (q*qscale)^T @ m0 needs no second pass over score.
  * Each core normalizes its mem block, computes G_c = mem_c^T @ mem_c
    [256,256] (bf16), AllGathers the 8 G_k (1MB).
  * Read phase sharded by embedding column: out[e,k,nq_shard,:] =
    emb_e[:,nq_shard]^T @ G_k (bf16 matmuls), bf16 output (upcast on host).
"""
import os
import sys

sys.path.insert(0, "/opt/trn_rl_repo")

import ml_dtypes
import numpy as np
import concourse.bacc as bacc
import concourse.mybir as mybir
import concourse.tile as tile
from concourse import masks
from concourse.bass_utils import run_bass_kernel_spmd

F32 = mybir.dt.float32
F32R = mybir.dt.float32r
BF16 = mybir.dt.bfloat16
ALU = mybir.AluOpType
AF = mybir.ActivationFunctionType

N, D, M_SLOTS, C, NQ = 8192, 256, 4608, 9, 16384
NCORES = 8
BLK = M_SLOTS // NCORES          # 576 slots per core
SHARD = NQ // NCORES             # 2048 embedding columns per core
NI = N // 128                    # 64 query chunks
JCH = [(0, 288), (288, 288)]     # j-chunks within the block (each own psum bank)
NT = (BLK + 127) // 128          # 5 slot tiles (4x128 + 64)
QG = 8                           # query chunks per batched qT DMA
EPS = 1e-12

_CACHE = {}


def _bcast_inner(ap, n):
    """Append a stride-0 inner dim of length n (broadcast along free dim)."""
    ap = ap.copy()
    ap.ap = ap.ap + [[0, n]]
    return ap


def _build():
    no_coll = os.environ.get("KNL_NO_COLL", "0") == "1"
    max_phase = int(os.environ.get("KNL_PHASE", "4"))
    repeat = int(os.environ.get("KNL_REPEAT", "1"))
    nc = bacc.Bacc("TRN2", target_bir_lowering=False, debug=False,
                   num_devices=NCORES)
    RG = [list(range(NCORES))]

    QT = nc.dram_tensor("qT", [2, D, N], BF16, kind="ExternalInput").ap()
    QB = nc.dram_tensor("qb", [N, D], BF16, kind="ExternalInput").ap()
    KBT = nc.dram_tensor("kbt", [2, 2, 128, BLK], BF16, kind="ExternalInput").ap()
    KBS = nc.dram_tensor("kbs", [NT * 128, D], F32, kind="ExternalInput").ap()
    EMB = nc.dram_tensor("emb", [2, D, SHARD], BF16, kind="ExternalInput").ap()
    RN = nc.dram_tensor("rnorm", [128, NI], F32, kind="ExternalInput").ap()
    ACTV = nc.dram_tensor("actv", [128, NT], F32, kind="ExternalInput").ap()
    OUT = nc.dram_tensor("out", [2, NCORES, SHARD, D], BF16,
                         kind="ExternalOutput").ap()
    debug = os.environ.get("KNL_DEBUG", "0") == "1"
    if debug:
        DBG_RMAX = nc.dram_tensor("dbg_rmax", [3, 128, NI], F32,
                                  kind="ExternalOutput").ap()
        DBG_CM = nc.dram_tensor("dbg_cm", [128, BLK], F32,
                                kind="ExternalOutput").ap()
        DBG_PRE = nc.dram_tensor("dbg_pre", [2, 128, NT], F32,
                                 kind="ExternalOutput").ap()
        DBG_QUT = nc.dram_tensor("dbg_qut", [2, 128, BLK], F32,
                                 kind="ExternalOutput").ap()
        DBG_MEM = nc.dram_tensor("dbg_mem", [128, NT, D], F32R,
                                 kind="ExternalOutput").ap()
        DBG_M0 = nc.dram_tensor("dbg_m0", [128, NI, BLK], BF16,
                                kind="ExternalOutput").ap()

    with tile.TileContext(nc) as tc:
        with (
            tc.tile_pool(name="const", bufs=1) as cp,
            tc.tile_pool(name="dram", bufs=1, space="DRAM") as dp,
        ):
            ident = cp.tile([128, 128], F32, tag="ident")
            masks.make_identity(nc, ident[:])

            rnorm = cp.tile([128, NI], F32, tag="rnorm")
            nc.sync.dma_start(rnorm[:], RN[:])
            actv = cp.tile([128, NT], F32, tag="actv")

            # keys*temp [p, t, d] for phase 3 (loaded mid-loop)
            kbs = cp.tile([128, NT, D], F32, tag="kbs", name="kbs")

            # ---- keysT [part][h][128, BLK]: bf16 hi/lo split (3-term
            # bf16 matmul gives fp32-grade scores at 1 cyc/row) ----
            keysT = [[cp.tile([128, BLK], BF16, tag=f"keysT{p_}{h}",
                              name=f"keysT{p_}{h}") for h in range(2)]
                     for p_ in range(2)]
            for p_ in range(2):
                for h in range(2):
                    nc.sync.dma_start(keysT[p_][h][:], KBT[p_, h])

            for rep in range(repeat):
                with tc.tile_pool(name=f"repc_r{rep}", bufs=1) as rp:
                    rmaxP = rp.tile([128, NI], F32, tag=f"rmaxP_r{rep}")
                    colmaxP = rp.tile([128, BLK], F32, tag=f"colmaxP_r{rep}")
                    m0 = rp.tile([128, NI, BLK], BF16, tag=f"m0_r{rep}",
                                 name=f"m0_r{rep}")
                    qres = rp.tile([128, NI, D], BF16, tag=f"qres_r{rep}",
                                   name=f"qres_r{rep}")
                    embr = [[rp.tile([128, SHARD], BF16, tag=f"embr{e}{h}_r{rep}",
                                     name=f"embr{e}{h}_r{rep}")
                             for h in range(2)] for e in range(2)]

                    nc.vector.memset(colmaxP[:], -1e30)

                    # ==== fused phase 1+2: score/maxes, split AR, qu accum ====
                    NAR = int(os.environ.get("KNL_NAR", "4"))
                    ARW = NI // NAR          # rows per AllReduce chunk
                    #  p2 trails p1 by LAG chunks (covers AR latency)
                    LAG = {4: 24, 2: 40, 1: 64}[NAR]
                    ar_in = [dp.tile([128, ARW], F32, tag=f"ar_in{k}_r{rep}",
                                     name=f"ar_in{k}_r{rep}") for k in range(NAR)]
                    ar_out = [dp.tile([128, ARW], F32, tag=f"ar_out{k}_r{rep}",
                                      name=f"ar_out{k}_r{rep}",
                                      addr_space=("Local" if no_coll else "Shared"))
                              for k in range(NAR)]
                    rmaxG = rp.tile([128, NI], F32, tag=f"rmaxG_r{rep}")
                    win = rp.tile([128, NI], F32, tag=f"win_r{rep}")
                    expp = rp.tile([128, NI], F32, tag=f"expp_r{rep}")
                    qscale = rp.tile([128, NI], F32, tag=f"qscale_r{rep}")
                    quT = [rp.tile([128, BLK], F32, tag=f"quT{h}_r{rep}",
                                   name=f"quT{h}_r{rep}") for h in range(2)]
                    qst = rp.tile([128, LAG, D], BF16, tag=f"qst_r{rep}",
                                  name=f"qst_r{rep}")

                    def emit_ar_dispatch(k):
                        lo, hi = k * ARW, (k + 1) * ARW
                        nc.sync.dma_start(ar_in[k][:], rmaxP[:, lo:hi])
                        if no_coll:
                            nc.sync.dma_start(ar_out[k].opt(), ar_in[k].opt())
                        else:
                            nc.gpsimd.collective_compute(
                                "AllReduce", ALU.max, replica_groups=RG,
                                ins=[ar_in[k].opt()], outs=[ar_out[k].opt()])
                        nc.sync.dma_start(rmaxG[:, lo:hi], ar_out[k][:])

                    def emit_ar_post(k):
                        # qscale = rnorm * exp(rmaxP) * (rmaxP >= rmaxG);
                        # deferred a few slots after dispatch so the rmaxG
                        # wait never head-of-line-blocks the DVE/ACT queues
                        lo, hi = k * ARW, (k + 1) * ARW
                        nc.vector.tensor_tensor(out=win[:, lo:hi],
                                                in0=rmaxP[:, lo:hi],
                                                in1=rmaxG[:, lo:hi], op=ALU.is_ge)
                        nc.scalar.activation(expp[:, lo:hi], rmaxP[:, lo:hi],
                                             AF.Exp)
                        nc.vector.tensor_tensor(out=win[:, lo:hi],
                                                in0=win[:, lo:hi],
                                                in1=expp[:, lo:hi], op=ALU.mult)
                        nc.vector.tensor_tensor(out=qscale[:, lo:hi],
                                                in0=win[:, lo:hi],
                                                in1=rnorm[:, lo:hi], op=ALU.mult)

                    with (
                        tc.tile_pool(name=f"p2sb_r{rep}", bufs=6) as p2,
                        tc.tile_pool(name=f"p2ps_r{rep}", bufs=1, space="PSUM") as pp2,
                    ):
                        pqu = [[pp2.tile([128, 288], F32, tag=f"pqu{h}{j}_r{rep}",
                                         name=f"pqu{h}{j}_r{rep}")
                                for j in range(2)] for h in range(2)]

                        def emit_p2(ic):
                            qs = p2.tile([128, D], BF16, tag="qs")
                            nc.scalar.activation(qs[:], qres[:, ic, :], AF.Copy,
                                                 scale=qscale[:, ic:ic + 1])
                            for h in range(2):
                                for j, (j0, jw) in enumerate(JCH):
                                    nc.tensor.matmul(
                                        pqu[h][j][:, 0:jw],
                                        qs[:, h * 128:(h + 1) * 128],
                                        m0[:, ic, j0:j0 + jw],
                                        start=(ic == 0), stop=(ic == NI - 1))

                        with (
                            tc.tile_pool(name=f"p1sb_r{rep}", bufs=2) as p1,
                            tc.tile_pool(name=f"p1sc_r{rep}", bufs=8) as p1s,
                            tc.tile_pool(name=f"p1ps_r{rep}", bufs=4, space="PSUM") as pp1,
                        ):
                            # first groups are small to prime the pipeline
                            GRP = [(0, 2), (2, 6)] + [(8 * g, 8)
                                                      for g in range(1, QG)]
                            g_of = {}
                            for gi, (st, sz) in enumerate(GRP):
                                for i in range(st, st + sz):
                                    g_of[i] = (gi, st, sz)

                            def emit_p1(ic):
                                gi, st, sz = g_of[ic]
                                r = ic - st
                                g = ic // QG
                                if r == 0:
                                    qq = [[p1.tile([128, QG * 128], BF16,
                                                   tag=f"q8{p_}{h}",
                                                   name=f"q8{p_}{h}_{ic}")
                                           for h in range(2)] for p_ in range(2)]
                                    for p_ in range(2):
                                        for h in range(2):
                                            nc.sync.dma_start(
                                                qq[p_][h][:, 0:sz * 128],
                                                QT[p_, h * 128:(h + 1) * 128,
                                                   st * 128:(st + sz) * 128])
                                    emit_p1.qq = qq
                                qq = emit_p1.qq
                                if ic % QG == 3:
                                    nc.sync.dma_start(
                                        qres[:, g * QG:(g + 1) * QG, :],
                                        QB[g * QG * 128:(g + 1) * QG * 128, :]
                                        .rearrange("(i p) d -> p i d", p=128))
                                if ic % QG == 5 and 1 <= g <= 4:
                                    e, h = divmod(g - 1, 2)
                                    nc.sync.dma_start(
                                        embr[e][h][:],
                                        EMB[e, h * 128:(h + 1) * 128, :])
                                if ic % QG == 6 and g == 5:
                                    nc.sync.dma_start(
                                        kbs[:],
                                        KBS.rearrange("(t p) d -> p t d", p=128))
                                    nc.sync.dma_start(actv[:], ACTV[:])
                                sc = p1s.tile([128, BLK], F32, tag="sc")
                                TERMS = [(0, 0), (0, 1), (1, 0)]  # (q, keys)
                                for (j0, jw) in JCH:
                                    ps = pp1.tile([128, 288], F32, tag="ps")
                                    for ti, (pq, pk) in enumerate(TERMS):
                                        for h in range(2):
                                            nc.tensor.matmul(
                                                ps[:, 0:jw],
                                                qq[pq][h][:, r * 128:(r + 1) * 128],
                                                keysT[pk][h][:, j0:j0 + jw],
                                                start=(ti == 0 and h == 0),
                                                stop=(ti == 2 and h == 1))
                                    nc.scalar.activation(sc[:, j0:j0 + jw],
                                                         ps[:, 0:jw], AF.Copy,
                                                         scale=rnorm[:, ic:ic + 1])
                                nc.vector.reduce_max(rmaxP[:, ic:ic + 1], sc[:],
                                                     axis=mybir.AxisListType.X)
                                # one-hot on GPSIMD (tensor_scalar+ptr is the
                                # only DVE-class op the Pool ISA accepts)
                                eng = (nc.gpsimd if os.environ.get(
                                    "KNL_POOL", "1") == "1" else nc.vector)
                                eng.tensor_scalar(
                                    out=m0[:, ic, :], in0=sc[:],
                                    scalar1=rmaxP[:, ic:ic + 1], scalar2=None,
                                    op0=ALU.is_ge)
                                nc.vector.tensor_tensor(
                                    out=colmaxP[:], in0=sc[:],
                                    in1=colmaxP[:], op=ALU.max)

                            for ic in range(NI):
                                emit_p1(ic)
                                if (ic + 1) % ARW == 0:
                                    emit_ar_dispatch((ic + 1) // ARW - 1)
                                if ic % ARW == min(ARW - 1, 3) and ic >= ARW:
                                    emit_ar_post(ic // ARW - 1)
                                if max_phase >= 2 and ic >= LAG:
                                    emit_p2(ic - LAG)
                        # colmax finalize overlaps the phase-2 tail
                        pre = rp.tile([128, NT], F32, tag=f"pre_r{rep}")
                        with tc.tile_pool(name=f"pcm_r{rep}", bufs=2,
                                          space="PSUM") as pcm:
                            cm = rp.tile([128, NT], F32, tag=f"cm_r{rep}")
                            nc.vector.memset(cm[:], 0.0)
                            for t in range(NT):
                                w = min(128, BLK - t * 128)
                                ps = pcm.tile([128, 128], F32, tag="ptc")
                                nc.tensor.transpose(ps[0:w, :],
                                                    colmaxP[:, t * 128:t * 128 + w],
                                                    ident[:])
                                nc.vector.reduce_max(cm[0:w, t:t + 1], ps[0:w, :],
                                                     axis=mybir.AxisListType.X)
                            # pre = active * exp(-colmax)  [128, NT]
                            emcm = rp.tile([128, NT], F32, tag=f"emcm_r{rep}")
                            nc.scalar.activation(emcm[:], cm[:], AF.Exp, scale=-1.0)
                            nc.vector.tensor_tensor(out=pre[:], in0=emcm[:],
                                                    in1=actv[:], op=ALU.mult)
                        if max_phase >= 2:
                            # tail: qs first (DVE, idle here), then bank-major
                            # matmuls so early banks release quT for phase 3
                            for ic in range(NI - LAG, NI):
                                if ic == NI - ARW:
                                    emit_ar_post(NAR - 1)
                                nc.vector.tensor_scalar_mul(
                                    qst[:, ic - (NI - LAG), :], qres[:, ic, :],
                                    qscale[:, ic:ic + 1])
                            for j, (j0, jw) in enumerate(JCH):
                                for h in range(2):
                                    for ic in range(NI - LAG, NI):
                                        nc.tensor.matmul(
                                            pqu[h][j][:, 0:jw],
                                            qst[:, ic - (NI - LAG),
                                                h * 128:(h + 1) * 128],
                                            m0[:, ic, j0:j0 + jw],
                                            start=False, stop=(ic == NI - 1))
                                    nc.scalar.copy(quT[h][:, j0:j0 + jw],
                                                   pqu[h][j][:, 0:jw])

                    if max_phase >= 3:
                        # ========= phase 3: mem block + G_c, AllGather G =========
                        gsb = rp.tile([128, 2, D], BF16, tag=f"gsb_r{rep}",
                                      name=f"gsb_r{rep}")
                        with (
                            tc.tile_pool(name=f"p3sb_r{rep}", bufs=1) as p3,
                            tc.tile_pool(name=f"p3ps_r{rep}", bufs=2, space="PSUM") as pp3,
                        ):
                            # qun [p, t, d] from quT via PE transposes,
                            # pre-scaled by `pre` during the PSUM->SBUF copy
                            qun = p3.tile([128, NT, D], F32, tag="qun")
                            nc.vector.memset(qun[:, NT - 1, :], 0.0)
                            for t in range(NT):
                                w = min(128, BLK - t * 128)
                                for h in range(2):
                                    ps = pp3.tile([128, 128], F32, tag="p3t")
                                    nc.tensor.transpose(ps[0:w, :],
                                                        quT[h][:, t * 128:t * 128 + w],
                                                        ident[:])
                                    nc.scalar.activation(
                                        qun[0:w, t, h * 128:(h + 1) * 128],
                                        ps[0:w, :], AF.Copy,
                                        scale=pre[0:w, t:t + 1])

                            # u = keys*temp + qun ; ss[t] = sum(u*u) per slot tile
                            u = p3.tile([128, NT, D], F32, tag="u")
                            usq = p3.tile([128, NT, D], F32, tag="usq")
                            ss = p3.tile([128, NT], F32, tag="ss")
                            for t in range(NT):
                                nc.vector.tensor_tensor(
                                    out=u[:, t, :], in0=qun[:, t, :],
                                    in1=kbs[:, t, :], op=ALU.add)
                                nc.vector.tensor_tensor(
                                    out=usq[:, t, :], in0=u[:, t, :],
                                    in1=u[:, t, :], op=ALU.mult)
                                nc.vector.reduce_sum(ss[:, t:t + 1], usq[:, t, :],
                                                     axis=mybir.AxisListType.X)
                            nc.vector.tensor_scalar_max(ss[:], ss[:], 1e-30)
                            # mem = u / sqrt(ss)  (rsqrt via ln/exp + Newton step)
                            lnv = p3.tile([128, NT], F32, tag="lnv")
                            nc.scalar.activation(lnv[:], ss[:], AF.Ln)
                            y = p3.tile([128, NT], F32, tag="y")
                            nc.scalar.activation(y[:], lnv[:], AF.Exp, scale=-0.5)
                            y2 = p3.tile([128, NT], F32, tag="y2")
                            nc.vector.tensor_tensor(out=y2[:], in0=y[:], in1=y[:],
                                                    op=ALU.mult)
                            sy2 = p3.tile([128, NT], F32, tag="sy2")
                            nc.vector.tensor_tensor(out=sy2[:], in0=ss[:], in1=y2[:],
                                                    op=ALU.mult)
                            corr = p3.tile([128, NT], F32, tag="corr")
                            nc.vector.tensor_scalar(
                                out=corr[:], in0=sy2[:], scalar1=-0.5,
                                scalar2=1.5, op0=ALU.mult, op1=ALU.add)
                            fac = p3.tile([128, NT], F32, tag="fac")
                            nc.vector.tensor_tensor(out=fac[:], in0=y[:],
                                                    in1=corr[:], op=ALU.mult)
                            memt = p3.tile([128, NT, D], F32R, tag="memt")
                            pg = [pp3.tile([128, D], F32, tag=f"pg{h}_r{rep}",
                                           name=f"pg{h}_r{rep}") for h in range(2)]
                            for t in range(NT):
                                w = min(128, BLK - t * 128)
                                nc.scalar.activation(memt[:, t, :], u[:, t, :],
                                                     AF.Copy,
                                                     scale=fac[:, t:t + 1])
                                for h in range(2):
                                    nc.tensor.matmul(
                                        pg[h][:],
                                        memt[0:w, t, h * 128:(h + 1) * 128],
                                        memt[0:w, t, :],
                                        start=(t == 0), stop=(t == NT - 1))
                            for h in range(2):
                                nc.scalar.copy(gsb[:, h, :], pg[h][:])

                        if debug:
                            nc.sync.dma_start(DBG_RMAX[0], rmaxP[:])
                            nc.sync.dma_start(DBG_RMAX[1], rmaxG[:])
                            nc.sync.dma_start(DBG_RMAX[2], qscale[:])
                            nc.sync.dma_start(DBG_CM[:], colmaxP[:])
                            nc.sync.dma_start(DBG_PRE[0], cm[:])
                            nc.sync.dma_start(DBG_PRE[1], pre[:])
                            for h in range(2):
                                nc.sync.dma_start(DBG_QUT[h], quT[h][:])
                            nc.sync.dma_start(DBG_MEM[:], memt[:])
                            nc.sync.dma_start(DBG_M0[:], m0[:])

                        g_in = dp.tile([D, D], BF16, tag=f"g_in_r{rep}")
                        g_out = dp.tile([NCORES, D, D], BF16, tag=f"g_out_r{rep}",
                                        addr_space=("Local" if no_coll else "Shared"))
                        nc.sync.dma_start(
                            g_in.rearrange("(h p) d -> p h d", p=128), gsb[:])
                        if no_coll:
                            nc.sync.dma_start(g_out[0], g_in[:])
                        else:
                            nc.gpsimd.collective_compute(
                                "AllGather", ALU.bypass, replica_groups=RG,
                                ins=[g_in.opt()], outs=[g_out.opt()])

                    if max_phase >= 4:
                        # ================= phase 4: reads =========================
                        with (
                            tc.tile_pool(name=f"p4sb_r{rep}", bufs=1) as p4c,
                            tc.tile_pool(name=f"p4out_r{rep}", bufs=4) as p4o,
                            tc.tile_pool(name=f"p4ps_r{rep}", bufs=2, space="PSUM") as pp4,
                        ):
                            gstt = p4c.tile([128, 2, NCORES, D], BF16,
                                            tag=f"gst_r{rep}", name=f"gst_r{rep}")
                            if no_coll:
                                for h in range(2):
                                    src = g_out[0, h * 128:(h + 1) * 128, :].copy()
                                    src.ap = src.ap[:1] + [[0, NCORES]] + src.ap[1:]
                                    nc.sync.dma_start(gstt[:, h, :, :], src)
                            else:
                                for h in range(2):
                                    nc.sync.dma_start(
                                        gstt[:, h, :, :],
                                        g_out[:, h * 128:(h + 1) * 128, :]
                                        .rearrange("k p d -> p k d"))
                            gst = [gstt[:, h, :, :].rearrange("p k d -> p (k d)")
                                   for h in range(2)]

                            for e in range(2):
                                for q in range(SHARD // 128):
                                    pso = [pp4.tile([128, 512], F32, tag=f"po{b}",
                                                    name=f"po{b}_{e}_{q}_r{rep}")
                                           for b in range(4)]
                                    for h in range(2):
                                        for b in range(4):
                                            nc.tensor.matmul(
                                                pso[b][:],
                                                embr[e][h][:, q * 128:(q + 1) * 128],
                                                gst[h][:, b * 512:(b + 1) * 512],
                                                start=(h == 0), stop=(h == 1))
                                    ob = p4o.tile([128, NCORES * D], BF16, tag="ob")
                                    for b in range(4):
                                        if b < 2:
                                            nc.scalar.copy(ob[:, b * 512:(b + 1) * 512],
                                                           pso[b][:])
                                        else:
                                            nc.vector.tensor_copy(
                                                ob[:, b * 512:(b + 1) * 512], pso[b][:])
                                    nc.sync.dma_start(
                                        OUT[e, :, q * 128:(q + 1) * 128, :]
                                        .rearrange("k p d -> p k d"),
                                        ob[:].rearrange("p (k d) -> p k d", k=NCORES))

    nc.compile()
    return nc


def _host_prep(query, keys, labels, class_counts):
    query = np.asarray(query, np.float32)
    rnorm = 1.0 / np.maximum(
        np.sqrt((query.astype(np.float32) ** 2).sum(1)), EPS)
    labels = np.asarray(labels)
    part = M_SLOTS // C
    slot_class = np.arange(M_SLOTS) // part
    active = np.isin(slot_class, labels).astype(np.float32)
    last = int(labels.max())
    cc = np.asarray(class_counts, np.float32)
    in_part = slot_class == last
    temp = np.where(in_part, cc[last], np.float32(1.0)).astype(np.float32)
    return rnorm.astype(np.float32), active, temp


def kernel(query, embeddings_src, embeddings_tgt, keys, class_counts,
           labels, num_classes, **_ignored):
    if "nc" not in _CACHE:
        _CACHE["nc"] = _build()
    nc = _CACHE["nc"]

    query = np.ascontiguousarray(np.asarray(query, np.float32))
    src = np.ascontiguousarray(np.asarray(embeddings_src, np.float32))
    tgt = np.ascontiguousarray(np.asarray(embeddings_tgt, np.float32))
    keys = np.ascontiguousarray(np.asarray(keys, np.float32))

    rnorm, active, temp = _host_prep(query, keys, labels, class_counts)

    def _split_bf16(a):
        hi = a.astype(ml_dtypes.bfloat16)
        lo = (a - hi.astype(np.float32)).astype(ml_dtypes.bfloat16)
        return hi, lo

    qTh, qTl = _split_bf16(np.ascontiguousarray(query.T))
    qT = np.ascontiguousarray(np.stack([qTh, qTl]))
    qb = query.astype(ml_dtypes.bfloat16)
    rn = np.ascontiguousarray(rnorm.reshape(NI, 128).T)
    keys_temp = keys * temp[:, None]

    in_maps = []
    for c in range(NCORES):
        sl = slice(c * SHARD, (c + 1) * SHARD)
        js = slice(c * BLK, (c + 1) * BLK)
        actv = np.zeros((128, NT), np.float32)
        for t in range(NT):
            w = min(128, BLK - t * 128)
            j0 = c * BLK + t * 128
            actv[0:w, t] = active[j0:j0 + w]
        kbs_pad = np.zeros((NT * 128, D), np.float32)
        kbs_pad[0:BLK] = keys_temp[js]
        kh, kl = _split_bf16(np.ascontiguousarray(keys[js].T))
        kbt = np.ascontiguousarray(
            np.stack([kh.reshape(2, 128, BLK), kl.reshape(2, 128, BLK)]))
        in_maps.append({
            "qT": qT,
            "qb": qb,
            "kbt": kbt,
            "kbs": kbs_pad,
            "emb": np.ascontiguousarray(
                np.stack([src[:, sl], tgt[:, sl]])).astype(ml_dtypes.bfloat16),
            "rnorm": rn,
            "actv": actv,
        })

    res = run_bass_kernel_spmd(nc, in_maps, list(range(NCORES)),
                               **_CACHE.get("run_kwargs", {}))
    _CACHE["last_result"] = res

    out = np.empty((2, NCORES, NQ, D), np.float32)
    for c in range(NCORES):
        out[:, :, c * SHARD:(c + 1) * SHARD, :] = res.results[c]["out"].astype(
            np.float32)
    return out


if __name__ == "__main__":
    import time
    os.environ.setdefault("JAX_PLATFORMS", "cpu")
    sys.path.insert(0, "/root/problem")
    import reference as R

    inputs = {k: (np.asarray(v) if not np.isscalar(v) else v)
              for k, v in R.setup_inputs().items()}
    t0 = time.time()
    got = kernel(**inputs)
    print(f"kernel wall (incl compile): {time.time()-t0:.1f}s")
    exp = np.load("expected.npy")
    scale = np.abs(exp).max()
    err = np.abs(got - exp)
    print("max abs err:", err.max(), " rel-to-absmax:", err.max() / scale)
    print("mean abs err:", err.mean())
